# revision 6
# baseline (speedup 1.0000x reference)
"""Multi-head causal attention (B=4,S=1024,D=768,H=12,Dh=64) on 8 trn2 cores.

Sharding: core c handles batch b=c//2 and the 6 heads hs=(c%2)*6 .. hs+6
(head-axis tensor parallel x batch parallel; 8 cores = 4 batches x 2 head-halves).

Per-core on-chip dataflow (all matmuls float32r, full PE rate at N>=256):
  xT [768,1024] (host-pretransposed), W{q,k,v} stacked [768, 384]
  qT/kT = W.T-chunks @ xT      -> [64,1024] per head (transposed layout)
  v     = xT-chunks.T @ Wv     -> [1024, 6*65] per t-chunk (65th col = ones)
  scoresT[t,s] tiles = kT-chunk.T(lhsT) x qT(rhs); causal tiles skipped,
  diag tiles masked by accumulating identity.T @ (-30000 lower-tri) in PSUM
  exp via ScalarE Exp(scale=1/8) straight from PSUM into a flat SBUF buffer
  ctxT_aug[65, s] = sum_j v_aug_j.T @ expT_j  (row 64 = softmax denominator)
  y_aug[h, 0:65, s] DMA'd out; host divides by denominators + transposes.
"""

import threading
from contextlib import ExitStack

import numpy as np

import concourse.bass as bass
import concourse.tile as tile
from concourse import bacc, mybir
from concourse.bass_utils import run_bass_kernel_spmd
from concourse.masks import make_identity

B, S, D, H, DH = 4, 1024, 768, 12, 64
NCORES = 8
HL = H // 2          # 6 local heads per core
KC = D // 128        # 6 contraction chunks
NPAIR = HL // 2      # head pairs for qk projection
F32R = mybir.dt.float32r
F32 = mybir.dt.float32
BF16 = mybir.dt.bfloat16
MASK_VAL = -30000.0


def _attn_groups():
    """Chunk table for one head's scoresT, packed into [128,1024] PSUM groups.

    A chunk (j, c) is the scoresT tile for t-chunk j (rows j*128..j*128+128)
    and s-range [s0, s0+w) inside output half c (s in [512c, 512c+512)).
    Only causal-relevant chunks exist. `diag` chunks need the triangular mask
    added to their first 128 columns. `ps_off` is the column offset inside the
    group's PSUM tile (each chunk stays inside one 512-col PSUM bank);
    `off` is the offset in the per-head flat exp buffer.
    """
    def chunk(j, c, ps_off):
        s0 = max(512 * c, 128 * j)
        w = 512 * (c + 1) - s0
        return dict(j=j, c=c, s0=s0, w=w, diag=(s0 == 128 * j), ps_off=ps_off)

    groups = [
        [chunk(0, 1, 0), chunk(0, 0, 512)],
        [chunk(1, 1, 0), chunk(1, 0, 512), chunk(7, 1, 896)],
        [chunk(2, 1, 0), chunk(2, 0, 512), chunk(6, 1, 768)],
        [chunk(3, 1, 0), chunk(3, 0, 512), chunk(5, 1, 640)],
        [chunk(4, 1, 0)],
    ]
    base = 0
    for g in groups:
        for ch in g:
            ch["off"] = base + ch["ps_off"]
        g_w = max(ch["ps_off"] + ch["w"] for ch in g)
        base += g_w
    total = base  # 4608
    return groups, total


def _emit_kernel(ctx: ExitStack, tc: tile.TileContext, xT, wq, wk, wv, y):
    nc = tc.nc
    groups, exp_cols = _attn_groups()

    const = ctx.enter_context(tc.tile_pool(name="const", bufs=1))
    ident = const.tile([128, 128], BF16)
    make_identity(nc, ident)
    mask = const.tile([128, 128], BF16)
    nc.gpsimd.memset(mask, 0.0)
    # mask[t, s] = 0 where s >= t else MASK_VAL (strict lower triangle in (t,s))
    nc.gpsimd.affine_select(
        out=mask, in_=mask,
        compare_op=mybir.AluOpType.is_ge,
        fill=MASK_VAL, base=0,
        pattern=[[1, 128]], channel_multiplier=-1,
    )

    qk_pool = ctx.enter_context(tc.tile_pool(name="qk", bufs=1))
    qT = qk_pool.tile([128, NPAIR, S], F32R)  # partitions: (h%2)*64+e, pair, s
    kT = qk_pool.tile([128, NPAIR, S], F32R)
    v_sb = qk_pool.tile([128, 8, HL * (DH + 1)], F32R)  # [t_rel, t_chunk, h*(65)]

    # ---------------- projections ----------------
    with tc.tile_pool(name="xtw", bufs=1) as xtw, \
         tc.tile_pool(name="pj", bufs=2, space="PSUM") as pj:
        xt = xtw.tile([128, KC, S], F32R)
        nc.sync.dma_start(out=xt, in_=xT[:, :].rearrange("(c p) s -> p c s", p=128))
        w_q = xtw.tile([128, KC, HL * DH], F32R)
        nc.sync.dma_start(out=w_q, in_=wq[:, :].rearrange("(c p) n -> p c n", p=128))
        w_k = xtw.tile([128, KC, HL * DH], F32R)
        nc.sync.dma_start(out=w_k, in_=wk[:, :].rearrange("(c p) n -> p c n", p=128))
        w_v = xtw.tile([128, KC, HL * DH], F32R)
        nc.sync.dma_start(out=w_v, in_=wv[:, :].rearrange("(c p) n -> p c n", p=128))

        for pp in range(NPAIR):
            for w_all, dst in ((w_q, qT), (w_k, kT)):
                ps0 = pj.tile([128, 512], F32, tag="pj0")
                ps1 = pj.tile([128, 512], F32, tag="pj1")
                for kc in range(KC):
                    for i, ps in enumerate((ps0, ps1)):
                        nc.tensor.matmul(
                            out=ps,
                            lhsT=w_all[:, kc, pp * 128:(pp + 1) * 128],
                            rhs=xt[:, kc, i * 512:(i + 1) * 512],
                            start=(kc == 0), stop=(kc == KC - 1),
                        )
                nc.scalar.copy(out=dst[:, pp, 0:512], in_=ps0)
                nc.scalar.copy(out=dst[:, pp, 512:1024], in_=ps1)

        for j in range(8):
            psv = pj.tile([128, HL * DH], F32, tag="pjv")
            for kc in range(KC):
                nc.tensor.matmul(
                    out=psv,
                    lhsT=xt[:, kc, j * 128:(j + 1) * 128],
                    rhs=w_v[:, kc, :],
                    start=(kc == 0), stop=(kc == KC - 1),
                )
            v_dst = v_sb[:, j, :].rearrange("p (h x) -> p h x", h=HL)
            nc.vector.tensor_copy(
                out=v_dst[:, :, 0:DH],
                in_=psv.rearrange("p (h e) -> p h e", h=HL),
            )
            # f32r memset is not a valid ISA encoding; write the bits as f32
            nc.vector.memset(v_dst[:, :, DH:DH + 1].bitcast(F32), 1.0)

    # ---------------- attention ----------------
    sg = ctx.enter_context(tc.tile_pool(name="sg", bufs=3, space="PSUM"))
    cx = ctx.enter_context(tc.tile_pool(name="cx", bufs=2, space="PSUM"))
    ex = ctx.enter_context(tc.tile_pool(name="ex", bufs=2))
    yst = ctx.enter_context(tc.tile_pool(name="yst", bufs=3))

    for h in range(HL):
        pair, half = h // 2, (h % 2) * 64
        qT_h = qT[half:half + 64, pair, :]
        kT_h = kT[half:half + 64, pair, :]

        exp_t = ex.tile([128, exp_cols], F32R, tag="exp")
        for g in groups:
            g_w = max(ch["ps_off"] + ch["w"] for ch in g)
            ps = sg.tile([128, 1024], F32, tag="sg")
            # One PSUM accumulation group per bank: first matmul in a bank
            # gets start=True, last gets stop=True; masks accumulate after
            # their bank's scores matmuls (identity stays loaded).
            for bank in (0, 1):
                ops = []
                for ch in g:  # j-sorted already; keeps kT loads adjacent
                    if ch["ps_off"] // 512 == bank:
                        ops.append(("mm", ch))
                for ch in g:
                    if ch["diag"] and ch["ps_off"] // 512 == bank:
                        ops.append(("mask", ch))
                for i, (kind, ch) in enumerate(ops):
                    first, last = (i == 0), (i == len(ops) - 1)
                    if kind == "mm":
                        nc.tensor.matmul(
                            out=ps[:, ch["ps_off"]:ch["ps_off"] + ch["w"]],
                            lhsT=kT_h[:, ch["j"] * 128:(ch["j"] + 1) * 128],
                            rhs=qT_h[:, ch["s0"]:ch["s0"] + ch["w"]],
                            start=first, stop=last,
                        )
                    else:
                        nc.tensor.matmul(
                            out=ps[:, ch["ps_off"]:ch["ps_off"] + 128],
                            lhsT=ident, rhs=mask,
                            start=first, stop=last,
                        )
            nc.scalar.activation(
                out=exp_t[:, g[0]["off"]:g[0]["off"] + g_w],
                in_=ps[:, 0:g_w],
                func=mybir.ActivationFunctionType.Exp,
                scale=1.0 / np.sqrt(DH),
            )

        chunks = [ch for g in groups for ch in g]
        for c in (0, 1):
            cc = sorted((ch for ch in chunks if ch["c"] == c), key=lambda t: t["j"])
            pc = cx.tile([DH + 1, 512], F32, tag="cx")
            for idx, ch in enumerate(cc):
                nc.tensor.matmul(
                    out=pc[:, ch["s0"] - 512 * c: ch["s0"] - 512 * c + ch["w"]],
                    lhsT=v_sb[:, ch["j"], :].rearrange(
                        "p (hh x) -> p hh x", hh=HL)[:, h, :],
                    rhs=exp_t[:, ch["off"]:ch["off"] + ch["w"]],
                    start=(idx == 0), stop=(idx == len(cc) - 1),
                )
            yt = yst.tile([DH + 1, 512], F32, tag="yst")
            nc.vector.tensor_copy(out=yt, in_=pc)
            nc.sync.dma_start(out=y[h, :, c * 512:(c + 1) * 512], in_=yt)


_PROGRAM = None
_PROGRAM_LOCK = threading.Lock()


def _get_program() -> bass.Bass:
    global _PROGRAM
    with _PROGRAM_LOCK:
        if _PROGRAM is None:
            nc = bacc.Bacc(None, target_bir_lowering=False)
            xT = nc.declare_dram_parameter("xT", [D, S], F32R, isOutput=False)
            wq = nc.declare_dram_parameter("wq", [D, HL * DH], F32R, isOutput=False)
            wk = nc.declare_dram_parameter("wk", [D, HL * DH], F32R, isOutput=False)
            wv = nc.declare_dram_parameter("wv", [D, HL * DH], F32R, isOutput=False)
            y = nc.declare_dram_parameter("y_aug", [HL, DH + 1, S], F32, isOutput=True)
            with tile.TileContext(nc) as tc, ExitStack() as ctx:
                _emit_kernel(ctx, tc, xT, wq, wk, wv, y)
            nc.finalize()  # runs Bacc passes (reg alloc, wait splitting)
            _PROGRAM = nc
    return _PROGRAM


def make_in_maps(x, Wq, Wk, Wv):
    """Per-core input dicts: batch b=core//2, heads (core%2)*6..+6."""
    in_maps = []
    for core in range(NCORES):
        b, hs = core // 2, (core % 2) * HL
        xTc = np.ascontiguousarray(x[b].T.astype(np.float32))
        maps = {"xT": xTc}
        for name, W in (("wq", Wq), ("wk", Wk), ("wv", Wv)):
            # [6,768,64] -> [768, 6*64], col = h*64+e
            maps[name] = np.ascontiguousarray(
                W[hs:hs + HL].transpose(1, 0, 2).reshape(D, HL * DH).astype(np.float32))
        in_maps.append(maps)
    return in_maps


def assemble_output(per_core_results):
    y_full = np.zeros((B, S, H * DH), np.float32)
    for core in range(NCORES):
        ya = per_core_results[core]["y_aug"]  # [6, 65, 1024]
        b, hs = core // 2, (core % 2) * HL
        ctxs = ya[:, 0:DH, :] / ya[:, DH:DH + 1, :]          # [6, 64, 1024]
        y_full[b, :, hs * DH:(hs + HL) * DH] = (
            ctxs.transpose(2, 0, 1).reshape(S, HL * DH))
    return y_full


def kernel(x, Wq, Wk, Wv):
    nc = _get_program()
    in_maps = make_in_maps(x, Wq, Wk, Wv)
    res = run_bass_kernel_spmd(nc, in_maps, core_ids=list(range(NCORES)))
    return assemble_output(res.results)


# revision 8
# speedup vs baseline: 1.3729x; 1.3729x over previous
"""Multi-head causal attention (B=4,S=1024,D=768,H=12,Dh=64) on 8 trn2 cores.

Sharding: core c handles batch b=c//2 and the 6 heads hs=(c%2)*6 .. hs+6
(head-axis tensor parallel x batch parallel; 8 cores = 4 batches x 2 head-halves).

Per-core on-chip dataflow (bf16 matmul operands, fp32 PSUM accumulation):
  xT [768,1024] (host-pretransposed bf16), W{q,k,v} stacked [768, 384] bf16
  qT/kT = W-chunk.T(lhsT) @ xT    -> [64,1024] per head (transposed layout)
  v     = xT-chunk.T @ Wv          -> [1024, 6*65] per t-chunk (65th col = ones)
  scoresT[t,s] tiles = kT-chunk(lhsT) x qT(rhs); fully-causal tiles skipped,
  diag tiles masked by accumulating identity @ (-30000 strict-lower-tri) in PSUM
  exp via ScalarE Exp(scale=1/8) straight from PSUM into a flat bf16 SBUF buffer
  ctxT_aug[65, s] = sum_j v_aug_j(lhsT) @ expT_j  (row 64 = softmax denominator)
  y_aug[h, 0:65, s] DMA'd out fp32; host divides by denominators + transposes.
"""

import threading
from contextlib import ExitStack

import ml_dtypes
import numpy as np

import concourse.bass as bass
import concourse.tile as tile
from concourse import bacc, mybir
from concourse.bass_utils import run_bass_kernel_spmd
from concourse.masks import make_identity

B, S, D, H, DH = 4, 1024, 768, 12, 64
NCORES = 8
HL = H // 2          # 6 local heads per core
KC = D // 128        # 6 contraction chunks
NPAIR = HL // 2      # head pairs for qk projection
F32 = mybir.dt.float32
BF16 = mybir.dt.bfloat16
MASK_VAL = -30000.0


def _attn_groups():
    """Chunk table for one head's scoresT, packed into [128,1024] PSUM groups.

    A chunk (j, c) is the scoresT tile for t-chunk j (rows j*128..j*128+128)
    and s-range [s0, s0+w) inside output half c (s in [512c, 512c+512)).
    Only causal-relevant chunks exist. `diag` chunks need the triangular mask
    added to their first 128 columns. `ps_off` is the column offset inside the
    group's PSUM tile (each chunk stays inside one 512-col PSUM bank);
    `off` is the offset in the per-head flat exp buffer.
    """
    def chunk(j, c, ps_off):
        s0 = max(512 * c, 128 * j)
        w = 512 * (c + 1) - s0
        return dict(j=j, c=c, s0=s0, w=w, diag=(s0 == 128 * j), ps_off=ps_off)

    groups = [
        [chunk(0, 1, 0), chunk(0, 0, 512)],
        [chunk(1, 1, 0), chunk(1, 0, 512), chunk(7, 1, 896)],
        [chunk(2, 1, 0), chunk(2, 0, 512), chunk(6, 1, 768)],
        [chunk(3, 1, 0), chunk(3, 0, 512), chunk(5, 1, 640)],
        [chunk(4, 1, 0)],
    ]
    base = 0
    for g in groups:
        for ch in g:
            ch["off"] = base + ch["ps_off"]
        g_w = max(ch["ps_off"] + ch["w"] for ch in g)
        base += g_w
    total = base  # 4608
    return groups, total


def _emit_kernel(ctx: ExitStack, tc: tile.TileContext, xT, wq, wk, wv, y):
    nc = tc.nc
    groups, exp_cols = _attn_groups()

    const = ctx.enter_context(tc.tile_pool(name="const", bufs=1))
    ident = const.tile([128, 128], BF16)
    make_identity(nc, ident)
    mask = const.tile([128, 128], BF16)
    nc.gpsimd.memset(mask, 0.0)
    # mask[t, s] = 0 where s >= t else MASK_VAL (strict lower triangle in (t,s))
    nc.gpsimd.affine_select(
        out=mask, in_=mask,
        compare_op=mybir.AluOpType.is_ge,
        fill=MASK_VAL, base=0,
        pattern=[[1, 128]], channel_multiplier=-1,
    )

    qk_pool = ctx.enter_context(tc.tile_pool(name="qk", bufs=1))
    qT = qk_pool.tile([128, NPAIR, S], BF16)  # partitions: (h%2)*64+e, pair, s
    kT = qk_pool.tile([128, NPAIR, S], BF16)
    v_sb = qk_pool.tile([128, 8, HL * (DH + 1)], BF16)  # [t_rel, t_chunk, h*65+x]

    # ---------------- projections ----------------
    with tc.tile_pool(name="xtw", bufs=1) as xtw, \
         tc.tile_pool(name="pj", bufs=1, space="PSUM") as pj:
        xt = xtw.tile([128, KC, S], BF16)
        w_q = xtw.tile([128, KC, HL * DH], BF16)
        w_k = xtw.tile([128, KC, HL * DH], BF16)
        w_v = xtw.tile([128, KC, HL * DH], BF16)
        # interleave per-chunk loads so projections start after chunk 0 lands
        for kc in range(KC):
            nc.sync.dma_start(out=xt[:, kc, :], in_=xT[kc * 128:(kc + 1) * 128, :])
            nc.sync.dma_start(out=w_q[:, kc, :], in_=wq[kc * 128:(kc + 1) * 128, :])
            nc.sync.dma_start(out=w_k[:, kc, :], in_=wk[kc * 128:(kc + 1) * 128, :])
        nc.sync.dma_start(out=w_v, in_=wv[:, :].rearrange("(c p) n -> p c n", p=128))

        # q then k: kc-outer accumulation into 6 resident PSUM tiles so the
        # first matmuls only need chunk-0 DMAs
        for w_all, dst in ((w_q, qT), (w_k, kT)):
            ps = [pj.tile([128, 512], F32, tag=f"pjq{t}", name=f"psq{t}")
                  for t in range(6)]
            for kc in range(KC):
                for pp in range(NPAIR):
                    for i in range(2):
                        nc.tensor.matmul(
                            out=ps[pp * 2 + i],
                            lhsT=w_all[:, kc, pp * 128:(pp + 1) * 128],
                            rhs=xt[:, kc, i * 512:(i + 1) * 512],
                            start=(kc == 0), stop=(kc == KC - 1),
                        )
            for pp in range(NPAIR):
                for i in range(2):
                    nc.vector.tensor_copy(
                        out=dst[:, pp, i * 512:(i + 1) * 512], in_=ps[pp * 2 + i])

        for j in range(8):
            psv = pj.tile([128, HL * DH], F32, tag=f"pjv{j % 2}")
            for kc in range(KC):
                nc.tensor.matmul(
                    out=psv,
                    lhsT=xt[:, kc, j * 128:(j + 1) * 128],
                    rhs=w_v[:, kc, :],
                    start=(kc == 0), stop=(kc == KC - 1),
                )
            v_dst = v_sb[:, j, :].rearrange("p (h x) -> p h x", h=HL)
            nc.vector.tensor_copy(
                out=v_dst[:, :, 0:DH],
                in_=psv.rearrange("p (h e) -> p h e", h=HL),
            )
            nc.vector.memset(v_dst[:, :, DH:DH + 1], 1.0)

    # ---------------- attention ----------------
    sg = ctx.enter_context(tc.tile_pool(name="sg", bufs=3, space="PSUM"))
    cx = ctx.enter_context(tc.tile_pool(name="cx", bufs=2, space="PSUM"))
    ex = ctx.enter_context(tc.tile_pool(name="ex", bufs=2))
    yst = ctx.enter_context(tc.tile_pool(name="yst", bufs=3))

    for h in range(HL):
        pair, half = h // 2, (h % 2) * 64
        qT_h = qT[half:half + 64, pair, :]
        kT_h = kT[half:half + 64, pair, :]

        exp_t = ex.tile([128, exp_cols], BF16, tag="exp")
        for g in groups:
            g_w = max(ch["ps_off"] + ch["w"] for ch in g)
            ps = sg.tile([128, 1024], F32, tag="sg")
            # One PSUM accumulation group per bank: first matmul in a bank
            # gets start=True, last gets stop=True; masks accumulate after
            # their bank's scores matmuls (identity stays loaded).
            for bank in (0, 1):
                ops = []
                for ch in g:  # j-sorted already; keeps kT loads adjacent
                    if ch["ps_off"] // 512 == bank:
                        ops.append(("mm", ch))
                for ch in g:
                    if ch["diag"] and ch["ps_off"] // 512 == bank:
                        ops.append(("mask", ch))
                for i, (kind, ch) in enumerate(ops):
                    first, last = (i == 0), (i == len(ops) - 1)
                    if kind == "mm":
                        nc.tensor.matmul(
                            out=ps[:, ch["ps_off"]:ch["ps_off"] + ch["w"]],
                            lhsT=kT_h[:, ch["j"] * 128:(ch["j"] + 1) * 128],
                            rhs=qT_h[:, ch["s0"]:ch["s0"] + ch["w"]],
                            start=first, stop=last,
                        )
                    else:
                        nc.tensor.matmul(
                            out=ps[:, ch["ps_off"]:ch["ps_off"] + 128],
                            lhsT=ident, rhs=mask,
                            start=first, stop=last,
                        )
            nc.scalar.activation(
                out=exp_t[:, g[0]["off"]:g[0]["off"] + g_w],
                in_=ps[:, 0:g_w],
                func=mybir.ActivationFunctionType.Exp,
                scale=1.0 / np.sqrt(DH),
            )

        chunks = [ch for g in groups for ch in g]
        for c in (0, 1):
            cc = sorted((ch for ch in chunks if ch["c"] == c), key=lambda t: t["j"])
            pc = cx.tile([DH + 1, 512], F32, tag="cx")
            for idx, ch in enumerate(cc):
                nc.tensor.matmul(
                    out=pc[:, ch["s0"] - 512 * c: ch["s0"] - 512 * c + ch["w"]],
                    lhsT=v_sb[:, ch["j"], :].rearrange(
                        "p (hh x) -> p hh x", hh=HL)[:, h, :],
                    rhs=exp_t[:, ch["off"]:ch["off"] + ch["w"]],
                    start=(idx == 0), stop=(idx == len(cc) - 1),
                )
            yt = yst.tile([DH + 1, 512], F32, tag="yst")
            nc.vector.tensor_copy(out=yt, in_=pc)
            nc.sync.dma_start(out=y[h, :, c * 512:(c + 1) * 512], in_=yt)


_PROGRAM = None
_PROGRAM_LOCK = threading.Lock()


def _get_program() -> bass.Bass:
    global _PROGRAM
    with _PROGRAM_LOCK:
        if _PROGRAM is None:
            nc = bacc.Bacc(None, target_bir_lowering=False)
            xT = nc.declare_dram_parameter("xT", [D, S], BF16, isOutput=False)
            wq = nc.declare_dram_parameter("wq", [D, HL * DH], BF16, isOutput=False)
            wk = nc.declare_dram_parameter("wk", [D, HL * DH], BF16, isOutput=False)
            wv = nc.declare_dram_parameter("wv", [D, HL * DH], BF16, isOutput=False)
            y = nc.declare_dram_parameter("y_aug", [HL, DH + 1, S], F32, isOutput=True)
            with tile.TileContext(nc) as tc, ExitStack() as ctx:
                _emit_kernel(ctx, tc, xT, wq, wk, wv, y)
            nc.finalize()  # runs Bacc passes (reg alloc, wait splitting)
            _PROGRAM = nc
    return _PROGRAM


def make_in_maps(x, Wq, Wk, Wv):
    """Per-core input dicts: batch b=core//2, heads (core%2)*6..+6."""
    bf = ml_dtypes.bfloat16
    in_maps = []
    for core in range(NCORES):
        b, hs = core // 2, (core % 2) * HL
        xTc = np.ascontiguousarray(np.asarray(x[b]).T.astype(bf))
        maps = {"xT": xTc}
        for name, W in (("wq", Wq), ("wk", Wk), ("wv", Wv)):
            # [6,768,64] -> [768, 6*64], col = h*64+e
            maps[name] = np.ascontiguousarray(
                np.asarray(W[hs:hs + HL]).transpose(1, 0, 2)
                .reshape(D, HL * DH).astype(bf))
        in_maps.append(maps)
    return in_maps


def assemble_output(per_core_results):
    y_full = np.zeros((B, S, H * DH), np.float32)
    for core in range(NCORES):
        ya = per_core_results[core]["y_aug"]  # [6, 65, 1024]
        b, hs = core // 2, (core % 2) * HL
        ctxs = ya[:, 0:DH, :] / ya[:, DH:DH + 1, :]          # [6, 64, 1024]
        y_full[b, :, hs * DH:(hs + HL) * DH] = (
            ctxs.transpose(2, 0, 1).reshape(S, HL * DH))
    return y_full


def kernel(x, Wq, Wk, Wv):
    nc = _get_program()
    in_maps = make_in_maps(x, Wq, Wk, Wv)
    res = run_bass_kernel_spmd(nc, in_maps, core_ids=list(range(NCORES)))
    return assemble_output(res.results)
